# revision 29
# baseline (speedup 1.0000x reference)
"""FBGCN layer on 8 Trainium2 NeuronCores — v9 (best measured 105.9us for
the v8 precursor vs 118us v3 baseline; v9 adds strictly-better overlap on
top; absolute numbers vary +-20% with machine load).

Math (reference):
    Lhp = (d_inv @ lap) @ d_inv
    Hh  = Lhp @ relu(x @ W_high)
    Hl  = GCNConv(x, edge_index, W_conv, b_conv)
    out = aL * Hl + aH * Hh

Structure (trace-driven; see _transcript for the measurements):
  * Exactly TWO AllGathers (P1 then P2).  Collectives here cost
    ~13us fixed + ~5.5us per 2MB gathered and consecutive collectives
    serialize, so splitting them (v4) regressed; two is the structural
    minimum for the d_inv@lap@d_inv@R chain with row sharding.
  * Stage B is m-pair-major: the pair {0,1} of PSUM tiles finishes at
    B's midpoint, so its PSUM->SBUF evacuation and bounce-buffer DMA
    overlap pair {2,3}'s matmuls and the AG1 doorbell fires right at
    B's end (the single biggest win, ~20us: the whole collective chain
    shifts left).
  * Staging evacuations alternate Scalar/Vector; v3 queued them behind
    20+ fp8 CASTs in the DVE FIFO, delaying the doorbell ~10us.
  * Stages C and E run in transposed form (out.T = V.T @ M.T): the thin
    operand becomes lhsT (natural layout, as delivered by the AG
    readback) and the wide matrix (aT / dT, host-pre-transposed) is the
    rhs with free dim 512 -> 64 matmuls per stage at ~98% PE column
    efficiency.  Output lands D-major [2*P, RPC]; host transposes.
    The bias becomes per-partition (bT) in transposed space.
  * C (fp8, deliberately plain-rate) and the second half of A_xw are
    the gap fillers: A_xw[0..12] + C[0..12] cover the AG1 flight;
    A_xw[13..31] + C[13..31] are fenced behind the AG2 staging tile
    (the deferred A_xw reads Whc2 = W_conv + 0*fence) so Tile cannot
    hoist them -- ~15us of work covers the AG2 flight and keeps the PE
    HAM-warm for E.
  * D's last chunk-block is m-pair-major with inline staging (same
    trick as B) so the AG2 doorbell fires right at D's end; E's last
    block is h-outer so the h=0 add+store overlaps h=1's matmuls.
  * Readback: small first piece (4 chunks, scalar ring) so D/E start
    early; the rest rides the then-idle sync ring in parallel.  D/E
    consume chunk-blocks matching the readback pieces.
  * Bulk loads in 8 large DMAs, need-order (W, x, d, l, a; l before a
    so no bulk traffic contends with AG1's SDMA data phase).
"""

import numpy as np
import ml_dtypes

import concourse.bass as bass
import concourse.mybir as mybir
import concourse.tile as tile
from concourse import bacc
from concourse.bass_utils import run_bass_kernel_spmd

N = 4096
D = 256
E = 131072
NCORES = 8
RPC = N // NCORES          # rows per core = 512
KC = N // 128              # contraction chunks = 32
MT = RPC // 128            # output row tiles per core = 4
P = 128

BF16 = mybir.dt.bfloat16
F32 = mybir.dt.float32
FP8 = mybir.dt.float8e4
nbf16 = ml_dtypes.bfloat16
nfp8 = ml_dtypes.float8_e4m3

RELU = mybir.ActivationFunctionType.Relu
COPY = mybir.ActivationFunctionType.Copy
IDENT = mybir.ActivationFunctionType.Identity

# readback halves: slots (= global chunks) 0..15 are ranks 0-3, 16..31 ranks 4-7
HALF1 = list(range(KC // 2))
HALF2 = list(range(KC // 2, KC))


def build_program(repeat: int = 1, ablate: frozenset = frozenset(), serial: bool = True):
    """Build the SPMD per-core program (identical on all cores)."""
    nc = bacc.Bacc(num_devices=NCORES)

    # ---- I/O ----  (matrix inputs come host-pre-transposed to [P, kc*m])
    xT = nc.declare_dram_parameter("xT", [P, 2 * N], BF16, isOutput=False)
    Whc = nc.declare_dram_parameter("Whc", [P, 2 * 2 * D], BF16, isOutput=False)
    dT = nc.declare_dram_parameter("dT", [P, KC * RPC], BF16, isOutput=False)
    lT = nc.declare_dram_parameter("lT", [P, KC * RPC], BF16, isOutput=False)
    aT = nc.declare_dram_parameter("aT", [P, KC * RPC], FP8, isOutput=False)
    bT = nc.declare_dram_parameter("bT", [P, 2], F32, isOutput=False)
    # transposed output: row = D coordinate (2 halves of 128), col = local row
    out = nc.declare_dram_parameter("out", [2 * P, RPC], BF16, isOutput=True)

    # collective bounce buffers: one gather = one collective of [P, MT*D]
    cc_in = {}
    cc_out = {}
    for g in (1, 2):
        cc_in[g] = nc.dram_tensor(f"cc{g}_in", [P, MT * D], BF16)
        cc_out[g] = nc.dram_tensor(
            f"cc{g}_out", [NCORES * P, MT * D], BF16, addr_space="Shared"
        )

    dT_v = dT.rearrange("p (kc m) -> p kc m", kc=KC)
    lT_v = lT.rearrange("p (kc m) -> p kc m", kc=KC)
    aT_v = aT.rearrange("p (kc m) -> p kc m", kc=KC)
    xT_v = xT.rearrange("p (kc m) -> p kc m", kc=2)
    Whc_v = Whc.rearrange("p (kc m) -> p kc m", kc=2)
    cc_in_v = {k: v.rearrange("p (mt m) -> p mt m", mt=MT) for k, v in cc_in.items()}
    # readback: rank r partition p holds chunks 4r..4r+3 as 4 contiguous D-cols
    cc_out_v = {
        k: v.rearrange("(rc p) (mt m) -> p rc mt m", p=P, mt=MT)
        for k, v in cc_out.items()
    }

    replica_groups = [list(range(NCORES))]

    def allgather(g):
        nc.gpsimd.collective_compute(
            "AllGather",
            mybir.AluOpType.bypass,
            replica_groups=replica_groups,
            ins=[cc_in[g][:]],
            outs=[cc_out[g][:]],
        )

    with tile.TileContext(nc) as tc:
        with (
            tc.tile_pool(name="const", bufs=1) as cpool,
            tc.tile_pool(name="bigmat", bufs=1) as bigpool,
            tc.tile_pool(name="acts", bufs=1) as apool,
            tc.tile_pool(name="psum", bufs=4, space="PSUM") as pspool,
            tc.tile_pool(name="outp", bufs=2) as opool,
        ):
            for _rep in range(repeat):
                if serial and _rep > 0:
                    # full flush between iterations: slope == single-shot latency
                    tc.strict_bb_all_engine_barrier()

                # ---- bulk loads, sync ring, few big DMAs, in need-order ----
                xT_sb = cpool.tile([P, 2, N], BF16, tag="xT")
                Whc_sb = cpool.tile([P, 2, 2 * D], BF16, tag="Whc")
                bT_sb = cpool.tile([P, 2], F32, tag="bT")
                d_sb = bigpool.tile([P, KC, RPC], BF16, tag="d")
                a_sb = bigpool.tile([P, KC, RPC], FP8, tag="a")
                l_sb = bigpool.tile([P, KC, RPC], BF16, tag="l")
                nc.sync.dma_start(out=Whc_sb[:], in_=Whc_v)
                # x in halves so stage A starts after ~1MB lands
                for mh in range(2):
                    s = slice(mh * (N // 2), (mh + 1) * (N // 2))
                    nc.sync.dma_start(out=xT_sb[:, :, s], in_=xT_v[:, :, s])
                if "load" not in ablate:
                    for c in range(2):
                        s = slice(c * (KC // 2), (c + 1) * (KC // 2))
                        nc.sync.dma_start(out=d_sb[:, s, :], in_=dT_v[:, s, :])
                    # l before a: l is needed at D (right after AG1) and
                    # loading it early keeps the SDMA engines free of bulk
                    # traffic during AG1's data phase
                    for c in range(2):
                        s = slice(c * (KC // 2), (c + 1) * (KC // 2))
                        nc.sync.dma_start(out=l_sb[:, s, :], in_=lT_v[:, s, :])
                    nc.sync.dma_start(out=a_sb[:], in_=aT_v)
                    nc.sync.dma_start(out=bT_sb[:], in_=bT[:])
                else:
                    nc.sync.dma_start(out=d_sb[:, :1, :64], in_=dT_v[:, :1, :64])
                    nc.sync.dma_start(out=a_sb[:, :1, :128], in_=aT_v[:, :1, :128])
                    nc.sync.dma_start(out=bT_sb[:], in_=bT[:])
                    nc.sync.dma_start(out=l_sb[:, :1, :64], in_=lT_v[:, :1, :64])

                # ---- stage A (R half): R = relu(x @ aH*W_high), bf16 ----
                R_sb = apool.tile([P, KC, D], BF16, tag="R")
                xw_sb = apool.tile([P, KC, D], FP8, tag="xw")
                if "A" in ablate:
                    nc.sync.dma_start(out=R_sb[:, :1, :64], in_=dT_v[:, :1, :64])
                    nc.sync.dma_start(out=xw_sb[:, :1, :128], in_=aT_v[:, :1, :128])
                if "A" not in ablate:
                    for m in range(KC):
                        psA = pspool.tile([P, D], F32, tag="ps", name=f"psA{m}_{_rep}")
                        for k in range(2):
                            nc.tensor.matmul(
                                out=psA[:],
                                lhsT=xT_sb[:, k, m * P:(m + 1) * P],
                                rhs=Whc_sb[:, k, :D],
                                start=(k == 0),
                                stop=(k == 1),
                            )
                        # alternate relu between Scalar (activation) and Vector
                        # (max with 0) so neither engine's ~0.37us/chunk
                        # serializes the A->B feed
                        if m % 2 == 0:
                            nc.scalar.activation(R_sb[:, m, :], psA[:], RELU)
                        else:
                            nc.vector.tensor_scalar_max(R_sb[:, m, :], psA[:], 0.0)

                def gather_store(g, psts):
                    # PSUM -> SBUF staging on Scalar, per-tile DMA so the first
                    # transfer starts while later tiles are still being copied
                    t = opool.tile([P, MT, D], BF16, tag="gst", name=f"gs{g}_{_rep}")
                    for m in range(MT):
                        # alternate copy engine so the 4 evacuations overlap
                        if m % 2 == 0:
                            nc.scalar.activation(t[:, m, :], psts[m][:], COPY)
                        else:
                            nc.vector.tensor_copy(t[:, m, :], psts[m][:])
                        nc.scalar.dma_start(out=cc_in_v[g][:, m, :], in_=t[:, m, :])
                    return t

                def gather_load(g, half, dst_sb):
                    # first piece small (4 chunks, scalar ring) so the consumer
                    # starts early; the rest rides the idle sync ring in
                    # parallel (bulk loads are done by now)
                    base = 0 if half == 1 else KC // 2
                    rc0 = 0 if half == 1 else NCORES // 2
                    for eng, (c0, c1) in ((nc.scalar, (0, 4)), (nc.sync, (4, 16))):
                        eng.dma_start(
                            out=dst_sb[:, base + c0:base + c1, :].rearrange(
                                "p (rc mt) m -> p rc mt m", mt=MT
                            ),
                            in_=cc_out_v[g][:, rc0 + c0 // MT:rc0 + c1 // MT, :, :],
                        )

                # ---- stage B (chunk-major): P1_loc = d_inv[rows] @ R ----
                if "B" not in ablate:
                    psB = {}
                    t1s = {}
                    for m in range(MT):
                        psB[m] = pspool.tile([P, D], F32, tag="ps", name=f"psB{m}_{_rep}")
                    # m-pair-major: pair {0,1} finishes at B's midpoint, so its
                    # staging copies+DMAs overlap pair {2,3}'s matmuls
                    t1 = opool.tile([P, MT, D], BF16, tag="gst", name=f"gs1_{_rep}")
                    for pair in (0, 1):
                        for c in range(KC):
                            for m in (2 * pair, 2 * pair + 1):
                                nc.tensor.matmul(
                                    out=psB[m][:],
                                    lhsT=d_sb[:, c, m * P:(m + 1) * P],
                                    rhs=R_sb[:, c, :],
                                    start=(c == 0),
                                    stop=(c == KC - 1),
                                )
                        for m in (2 * pair, 2 * pair + 1):
                            if m % 2 == 0:
                                nc.scalar.activation(t1[:, m, :], psB[m][:], COPY)
                            else:
                                nc.vector.tensor_copy(t1[:, m, :], psB[m][:])
                            nc.scalar.dma_start(
                                out=cc_in_v[1][:, m, :], in_=t1[:, m, :]
                            )
                    if "AG1" not in ablate:
                        allgather(1)

                # ---- stage A (xw half, deferred): xw = fp8(x @ W_conv) ----
                # chunks 0..12 cover the AG1 flight; 13..31 are fenced into
                # the AG2 window below (more fill there, keeps PE warm for E)
                CSPLIT = 13

                def a_xw_chunks(ms, whc):
                    for m in ms:
                        psX = pspool.tile([P, D], F32, tag="ps", name=f"psX{m}_{_rep}")
                        for k in range(2):
                            nc.tensor.matmul(
                                out=psX[:],
                                lhsT=xT_sb[:, k, m * P:(m + 1) * P],
                                rhs=whc[:, k, :],
                                start=(k == 0),
                                stop=(k == 1),
                            )
                        nc.vector.tensor_copy(xw_sb[:, m, :], psX[:])

                if "A" not in ablate:
                    a_xw_chunks(range(CSPLIT), Whc_sb[:, :, D:])

                # ---- stage C (transposed): HlT = (A_T.T @ xw).T accumulation
                # psC[h] [P,512] += xw[:,c,h*128:].T @ aT[:,c,:]  (fp8, N=512)
                psC = {}
                if "C" not in ablate:
                    for h in range(2):
                        psC[h] = pspool.tile(
                            [P, RPC], F32, tag="psw", bufs=4, name=f"psC{h}_{_rep}"
                        )

                def stage_c_chunks(cs, first, last, lhs=None, off=0):
                    src = xw_sb if lhs is None else lhs
                    for c in cs:
                        for h in range(2):
                            nc.tensor.matmul(
                                out=psC[h][:],
                                lhsT=src[:, c - off, h * P:(h + 1) * P],
                                rhs=a_sb[:, c, :],
                                start=(first and c == cs[0]),
                                stop=(last and c == cs[-1]),
                            )

                if "C" not in ablate:
                    stage_c_chunks(list(range(CSPLIT)), True, False)

                # ---- stage D: P2_loc = lap[rows] @ P1, split on rb halves ----
                P1_sb = apool.tile([P, KC, D], BF16, tag="P1")
                gather_load(1, 1, P1_sb)
                gather_load(1, 2, P1_sb)
                psD = {}
                if "D" not in ablate:
                    for m in range(MT):
                        psD[m] = pspool.tile([P, D], F32, tag="ps", name=f"psD{m}_{_rep}")
                    # chunk-block-major: consume each readback piece for all m
                    # before needing the next piece; the LAST block runs
                    # m-pair-major so pair {0,1}'s staging overlaps pair
                    # {2,3}'s matmuls and the AG2 doorbell fires at D's end
                    blocks = [HALF1[:4], HALF1[4:], HALF2[:4], HALF2[4:]]
                    for bi, blk in enumerate(blocks[:3]):
                        for m in range(MT):
                            for c in blk:
                                nc.tensor.matmul(
                                    out=psD[m][:],
                                    lhsT=l_sb[:, c, m * P:(m + 1) * P],
                                    rhs=P1_sb[:, c, :],
                                    start=(bi == 0 and c == blk[0]),
                                    stop=False,
                                )
                    blk = blocks[3]
                    t2 = opool.tile([P, MT, D], BF16, tag="gst", name=f"gs2_{_rep}")
                    for pair in (0, 1):
                        for m in (2 * pair, 2 * pair + 1):
                            for c in blk:
                                nc.tensor.matmul(
                                    out=psD[m][:],
                                    lhsT=l_sb[:, c, m * P:(m + 1) * P],
                                    rhs=P1_sb[:, c, :],
                                    start=False,
                                    stop=(c == blk[-1]),
                                )
                        for m in (2 * pair, 2 * pair + 1):
                            if m % 2 == 0:
                                nc.scalar.activation(t2[:, m, :], psD[m][:], COPY)
                            else:
                                nc.vector.tensor_copy(t2[:, m, :], psD[m][:])
                            nc.scalar.dma_start(
                                out=cc_in_v[2][:, m, :], in_=t2[:, m, :]
                            )
                    if "AG2" not in ablate:
                        allgather(2)

                # ---- stage C (rest): covers AG2 flight; fold bias on evacuate
                # fence: xw2 = xw[16:] + 0, where the 0 is derived from the AG2
                # staging tile -- a data dependency that stops Tile from
                # hoisting these matmuls ahead of the AG2 doorbell (v5 ran all
                # of C early, leaving the AG2 flight uncovered and E cold)
                HlT_sb = opool.tile([P, 2, RPC], BF16, tag="HlT")
                if "A" not in ablate and "D" not in ablate:
                    # fence: a zero derived from the AG2 staging tile gates a
                    # copy of W_conv; the deferred A_xw chunks read it, so
                    # Tile cannot hoist them (or the C chunks that consume
                    # their xw output) ahead of the AG2 doorbell
                    fence_t = opool.tile([P, 1], F32, tag="fence")
                    nc.vector.tensor_scalar_mul(fence_t[:], t2[:, MT - 1, :1], 0.0)
                    Whc2_sb = cpool.tile([P, 2, D], BF16, tag="Whc2")
                    nc.vector.tensor_scalar_add(
                        Whc2_sb[:], Whc_sb[:, :, D:], fence_t[:]
                    )
                    a_xw_chunks(range(CSPLIT, KC), Whc2_sb)
                elif "A" not in ablate:
                    a_xw_chunks(range(CSPLIT, KC), Whc_sb[:, :, D:])
                if "C" not in ablate:
                    stage_c_chunks(list(range(CSPLIT, KC)), False, True)
                    for h in range(2):
                        nc.scalar.activation(
                            HlT_sb[:, h, :], psC[h][:], IDENT, bias=bT_sb[:, h:h + 1]
                        )
                else:
                    for h in range(2):
                        nc.vector.memset(HlT_sb[:, h, :], 0.0)

                # ---- stage E (transposed): HhT[h] += P2[:,c,h].T @ dT[:,c,:]
                P2_sb = apool.tile([P, KC, D], BF16, tag="P2")
                gather_load(2, 1, P2_sb)
                gather_load(2, 2, P2_sb)
                if "E" not in ablate:
                    psE = {}
                    for h in range(2):
                        psE[h] = pspool.tile(
                            [P, RPC], F32, tag="psw", bufs=4, name=f"psE{h}_{_rep}"
                        )
                    # last block h-outer: h=0 finishes 12 matmuls early, so
                    # its add+store overlaps h=1's tail
                    blocks = [HALF1[:4], HALF1[4:], HALF2[:4], HALF2[4:]]
                    for bi, blk in enumerate(blocks[:3]):
                        for c in blk:
                            for h in range(2):
                                nc.tensor.matmul(
                                    out=psE[h][:],
                                    lhsT=P2_sb[:, c, h * P:(h + 1) * P],
                                    rhs=d_sb[:, c, :],
                                    start=(bi == 0 and c == blk[0]),
                                    stop=False,
                                )
                    blk = blocks[3]
                    for h in range(2):
                        for c in blk:
                            nc.tensor.matmul(
                                out=psE[h][:],
                                lhsT=P2_sb[:, c, h * P:(h + 1) * P],
                                rhs=d_sb[:, c, :],
                                start=False,
                                stop=(c == blk[-1]),
                            )
                        o_sb = opool.tile([P, RPC], BF16, tag="osb", name=f"os{h}_{_rep}")
                        nc.vector.tensor_add(o_sb[:], psE[h][:], HlT_sb[:, h, :])
                        nc.scalar.dma_start(out=out[h * P:(h + 1) * P, :], in_=o_sb[:])

    nc.finalize()
    return nc


def prep_inputs(x, edge_index, lap, d_inv, W_high, W_conv, b_conv, aL, aH):
    """Host-side sharding/layout: build per-core input maps."""
    x = np.asarray(x, dtype=np.float32)
    lap = np.asarray(lap, dtype=np.float32)
    d_inv = np.asarray(d_inv, dtype=np.float32)
    W_high = np.asarray(W_high, dtype=np.float32)
    W_conv = np.asarray(W_conv, dtype=np.float32)
    b_conv = np.asarray(b_conv, dtype=np.float32)
    aLs = float(np.asarray(aL).reshape(-1)[0])
    aHs = float(np.asarray(aH).reshape(-1)[0])
    src = np.asarray(edge_index[0], dtype=np.int64)
    dst = np.asarray(edge_index[1], dtype=np.int64)

    # symmetric GCN normalization (with self-loops) folded into a dense adjacency
    deg = np.bincount(dst, minlength=N).astype(np.float32) + 1.0
    dis = 1.0 / np.sqrt(deg)
    A_T = np.zeros((N, N), dtype=np.float32)           # A_T[src, dst]
    np.add.at(A_T, (src, dst), aLs * dis[src] * dis[dst])
    A_T[np.arange(N), np.arange(N)] += aLs * dis * dis

    def to_pkm(arrT):
        # [K, M] -> [P, kc*M]: element (p, c*M + m) = arrT[128*c + p, m]
        Kdim, Mdim = arrT.shape
        kc = Kdim // P
        a = arrT.reshape(kc, P, Mdim)
        return np.ascontiguousarray(a.transpose(1, 0, 2).reshape(P, kc * Mdim))

    xT = to_pkm(np.ascontiguousarray(x.T).astype(nbf16))
    Whc = to_pkm(np.concatenate([W_high * aHs, W_conv], axis=1).astype(nbf16))
    # bias along D (partition axis in transposed space): [P, 2] f32
    bTm = np.ascontiguousarray((aLs * b_conv).reshape(2, P).T).astype(np.float32)
    dT_full = np.ascontiguousarray(d_inv.T).astype(nbf16)
    lT_full = np.ascontiguousarray(lap.T).astype(nbf16)
    aT_full = np.clip(A_T, -240, 240).astype(nfp8)

    in_maps = []
    for i in range(NCORES):
        sl = slice(i * RPC, (i + 1) * RPC)
        in_maps.append({
            "xT": xT,
            "Whc": Whc,
            "dT": to_pkm(dT_full[:, sl]),
            "lT": to_pkm(lT_full[:, sl]),
            "aT": to_pkm(aT_full[:, sl]),
            "bT": bTm,
        })
    return in_maps


def kernel(x, edge_index, lap, d_inv, W_high, W_conv, b_conv, aL, aH):
    in_maps = prep_inputs(x, edge_index, lap, d_inv, W_high, W_conv, b_conv, aL, aH)
    nc = build_program()
    res = run_bass_kernel_spmd(nc, in_maps, list(range(NCORES)))
    # per-core output is D-major [256, 512]; transpose back to [512, 256]
    return np.concatenate(
        [res.results[i]["out"].T for i in range(NCORES)], axis=0
    ).astype(np.float32)


# revision 31
# speedup vs baseline: 1.0228x; 1.0228x over previous
"""FBGCN layer on 8 Trainium2 NeuronCores — v9 (best measured 105.9us for
the v8 precursor vs 118us v3 baseline; v9 adds strictly-better overlap on
top; absolute numbers vary +-20% with machine load).

Math (reference):
    Lhp = (d_inv @ lap) @ d_inv
    Hh  = Lhp @ relu(x @ W_high)
    Hl  = GCNConv(x, edge_index, W_conv, b_conv)
    out = aL * Hl + aH * Hh

Structure (trace-driven; see _transcript for the measurements):
  * Exactly TWO AllGathers (P1 then P2).  Collectives here cost
    ~13us fixed + ~5.5us per 2MB gathered and consecutive collectives
    serialize, so splitting them (v4) regressed; two is the structural
    minimum for the d_inv@lap@d_inv@R chain with row sharding.
  * Stage B is m-pair-major: the pair {0,1} of PSUM tiles finishes at
    B's midpoint, so its PSUM->SBUF evacuation and bounce-buffer DMA
    overlap pair {2,3}'s matmuls and the AG1 doorbell fires right at
    B's end (the single biggest win, ~20us: the whole collective chain
    shifts left).
  * Staging evacuations alternate Scalar/Vector; v3 queued them behind
    20+ fp8 CASTs in the DVE FIFO, delaying the doorbell ~10us.
  * Stages C and E run in transposed form (out.T = V.T @ M.T): the thin
    operand becomes lhsT (natural layout, as delivered by the AG
    readback) and the wide matrix (aT / dT, host-pre-transposed) is the
    rhs with free dim 512 -> 64 matmuls per stage at ~98% PE column
    efficiency.  Output lands D-major [2*P, RPC]; host transposes.
    The bias becomes per-partition (bT) in transposed space.
  * C (fp8, deliberately plain-rate) and the second half of A_xw are
    the gap fillers: A_xw[0..12] + C[0..12] cover the AG1 flight;
    A_xw[13..31] + C[13..31] are fenced behind the AG2 staging tile
    (the deferred A_xw reads Whc2 = W_conv + 0*fence) so Tile cannot
    hoist them -- ~15us of work covers the AG2 flight and keeps the PE
    HAM-warm for E.
  * D's last chunk-block is m-pair-major with inline staging (same
    trick as B) so the AG2 doorbell fires right at D's end; E's last
    block is h-outer so the h=0 add+store overlaps h=1's matmuls.
  * Readback: small first piece (4 chunks, scalar ring) so D/E start
    early; the rest rides the then-idle sync ring in parallel.  D/E
    consume chunk-blocks matching the readback pieces.
  * Bulk loads in 8 large DMAs, need-order (W, x, d, l, a; l before a
    so no bulk traffic contends with AG1's SDMA data phase).
"""

import numpy as np
import ml_dtypes

import concourse.bass as bass
import concourse.mybir as mybir
import concourse.tile as tile
from concourse import bacc
from concourse.bass_utils import run_bass_kernel_spmd

N = 4096
D = 256
E = 131072
NCORES = 8
RPC = N // NCORES          # rows per core = 512
KC = N // 128              # contraction chunks = 32
MT = RPC // 128            # output row tiles per core = 4
P = 128

BF16 = mybir.dt.bfloat16
F32 = mybir.dt.float32
FP8 = mybir.dt.float8e4
nbf16 = ml_dtypes.bfloat16
nfp8 = ml_dtypes.float8_e4m3

RELU = mybir.ActivationFunctionType.Relu
COPY = mybir.ActivationFunctionType.Copy
IDENT = mybir.ActivationFunctionType.Identity

# readback halves: slots (= global chunks) 0..15 are ranks 0-3, 16..31 ranks 4-7
HALF1 = list(range(KC // 2))
HALF2 = list(range(KC // 2, KC))


def build_program(repeat: int = 1, ablate: frozenset = frozenset(), serial: bool = True):
    """Build the SPMD per-core program (identical on all cores)."""
    nc = bacc.Bacc(num_devices=NCORES)

    # ---- I/O ----  (matrix inputs come host-pre-transposed to [P, kc*m])
    xT = nc.declare_dram_parameter("xT", [P, 2 * N], BF16, isOutput=False)
    Whc = nc.declare_dram_parameter("Whc", [P, 2 * 2 * D], BF16, isOutput=False)
    dT = nc.declare_dram_parameter("dT", [P, KC * RPC], BF16, isOutput=False)
    lT = nc.declare_dram_parameter("lT", [P, KC * RPC], BF16, isOutput=False)
    aT = nc.declare_dram_parameter("aT", [P, KC * RPC], FP8, isOutput=False)
    bT = nc.declare_dram_parameter("bT", [P, 2], F32, isOutput=False)
    # transposed output: row = D coordinate (2 halves of 128), col = local row
    out = nc.declare_dram_parameter("out", [2 * P, RPC], BF16, isOutput=True)

    # collective bounce buffers: one gather = one collective of [P, MT*D]
    cc_in = {}
    cc_out = {}
    for g in (1, 2):
        cc_in[g] = nc.dram_tensor(f"cc{g}_in", [P, MT * D], BF16)
        cc_out[g] = nc.dram_tensor(
            f"cc{g}_out", [NCORES * P, MT * D], BF16, addr_space="Shared"
        )

    dT_v = dT.rearrange("p (kc m) -> p kc m", kc=KC)
    lT_v = lT.rearrange("p (kc m) -> p kc m", kc=KC)
    aT_v = aT.rearrange("p (kc m) -> p kc m", kc=KC)
    xT_v = xT.rearrange("p (kc m) -> p kc m", kc=2)
    Whc_v = Whc.rearrange("p (kc m) -> p kc m", kc=2)
    cc_in_v = {k: v.rearrange("p (mt m) -> p mt m", mt=MT) for k, v in cc_in.items()}
    # readback: rank r partition p holds chunks 4r..4r+3 as 4 contiguous D-cols
    cc_out_v = {
        k: v.rearrange("(rc p) (mt m) -> p rc mt m", p=P, mt=MT)
        for k, v in cc_out.items()
    }

    replica_groups = [list(range(NCORES))]

    def allgather(g):
        nc.gpsimd.collective_compute(
            "AllGather",
            mybir.AluOpType.bypass,
            replica_groups=replica_groups,
            ins=[cc_in[g][:]],
            outs=[cc_out[g][:]],
        )

    with tile.TileContext(nc) as tc:
        with (
            tc.tile_pool(name="const", bufs=1) as cpool,
            tc.tile_pool(name="bigmat", bufs=1) as bigpool,
            tc.tile_pool(name="acts", bufs=1) as apool,
            tc.tile_pool(name="psum", bufs=4, space="PSUM") as pspool,
            tc.tile_pool(name="outp", bufs=2) as opool,
        ):
            for _rep in range(repeat):
                if serial and _rep > 0:
                    # full flush between iterations: slope == single-shot latency
                    tc.strict_bb_all_engine_barrier()

                # ---- bulk loads, sync ring, few big DMAs, in need-order ----
                xT_sb = cpool.tile([P, 2, N], BF16, tag="xT")
                Whc_sb = cpool.tile([P, 2, 2 * D], BF16, tag="Whc")
                bT_sb = cpool.tile([P, 2], F32, tag="bT")
                d_sb = bigpool.tile([P, KC, RPC], BF16, tag="d")
                a_sb = bigpool.tile([P, KC, RPC], FP8, tag="a")
                l_sb = bigpool.tile([P, KC, RPC], BF16, tag="l")
                nc.sync.dma_start(out=Whc_sb[:], in_=Whc_v)
                # x in halves so stage A starts after ~1MB lands
                for mh in range(2):
                    s = slice(mh * (N // 2), (mh + 1) * (N // 2))
                    nc.sync.dma_start(out=xT_sb[:, :, s], in_=xT_v[:, :, s])
                if "load" not in ablate:
                    for c in range(2):
                        s = slice(c * (KC // 2), (c + 1) * (KC // 2))
                        nc.sync.dma_start(out=d_sb[:, s, :], in_=dT_v[:, s, :])
                    # l before a: l is needed at D (right after AG1) and
                    # loading it early keeps the SDMA engines free of bulk
                    # traffic during AG1's data phase
                    for c in range(2):
                        s = slice(c * (KC // 2), (c + 1) * (KC // 2))
                        nc.sync.dma_start(out=l_sb[:, s, :], in_=lT_v[:, s, :])
                    nc.sync.dma_start(out=a_sb[:], in_=aT_v)
                    nc.sync.dma_start(out=bT_sb[:], in_=bT[:])
                else:
                    nc.sync.dma_start(out=d_sb[:, :1, :64], in_=dT_v[:, :1, :64])
                    nc.sync.dma_start(out=a_sb[:, :1, :128], in_=aT_v[:, :1, :128])
                    nc.sync.dma_start(out=bT_sb[:], in_=bT[:])
                    nc.sync.dma_start(out=l_sb[:, :1, :64], in_=lT_v[:, :1, :64])

                # ---- stage A (R half): R = relu(x @ aH*W_high), bf16 ----
                R_sb = apool.tile([P, KC, D], BF16, tag="R")
                xw_sb = apool.tile([P, KC, D], FP8, tag="xw")
                if "A" in ablate:
                    nc.sync.dma_start(out=R_sb[:, :1, :64], in_=dT_v[:, :1, :64])
                    nc.sync.dma_start(out=xw_sb[:, :1, :128], in_=aT_v[:, :1, :128])
                if "A" not in ablate:
                    for m in range(KC):
                        psA = pspool.tile([P, D], F32, tag="ps", name=f"psA{m}_{_rep}")
                        for k in range(2):
                            nc.tensor.matmul(
                                out=psA[:],
                                lhsT=xT_sb[:, k, m * P:(m + 1) * P],
                                rhs=Whc_sb[:, k, :D],
                                start=(k == 0),
                                stop=(k == 1),
                            )
                        # alternate relu between Scalar (activation) and Vector
                        # (max with 0) so neither engine's ~0.37us/chunk
                        # serializes the A->B feed
                        if m % 2 == 0:
                            nc.scalar.activation(R_sb[:, m, :], psA[:], RELU)
                        else:
                            nc.vector.tensor_scalar_max(R_sb[:, m, :], psA[:], 0.0)

                def gather_store(g, psts):
                    # PSUM -> SBUF staging on Scalar, per-tile DMA so the first
                    # transfer starts while later tiles are still being copied
                    t = opool.tile([P, MT, D], BF16, tag="gst", name=f"gs{g}_{_rep}")
                    for m in range(MT):
                        # alternate copy engine so the 4 evacuations overlap
                        if m % 2 == 0:
                            nc.scalar.activation(t[:, m, :], psts[m][:], COPY)
                        else:
                            nc.vector.tensor_copy(t[:, m, :], psts[m][:])
                        nc.scalar.dma_start(out=cc_in_v[g][:, m, :], in_=t[:, m, :])
                    return t

                def gather_load(g, half, dst_sb):
                    # first piece small (4 chunks, scalar ring) so the consumer
                    # starts early; the rest rides the idle sync ring in
                    # parallel (bulk loads are done by now)
                    base = 0 if half == 1 else KC // 2
                    rc0 = 0 if half == 1 else NCORES // 2
                    for eng, (c0, c1) in ((nc.scalar, (0, 4)), (nc.sync, (4, 16))):
                        eng.dma_start(
                            out=dst_sb[:, base + c0:base + c1, :].rearrange(
                                "p (rc mt) m -> p rc mt m", mt=MT
                            ),
                            in_=cc_out_v[g][:, rc0 + c0 // MT:rc0 + c1 // MT, :, :],
                        )

                # ---- stage B (chunk-major): P1_loc = d_inv[rows] @ R ----
                if "B" not in ablate:
                    psB = {}
                    t1s = {}
                    for m in range(MT):
                        psB[m] = pspool.tile([P, D], F32, tag="ps", name=f"psB{m}_{_rep}")
                    # m-pair-major: pair {0,1} finishes at B's midpoint, so its
                    # staging copies+DMAs overlap pair {2,3}'s matmuls
                    t1 = opool.tile([P, MT, D], BF16, tag="gst", name=f"gs1_{_rep}")
                    for pair in (0, 1):
                        for c in range(KC):
                            for m in (2 * pair, 2 * pair + 1):
                                nc.tensor.matmul(
                                    out=psB[m][:],
                                    lhsT=d_sb[:, c, m * P:(m + 1) * P],
                                    rhs=R_sb[:, c, :],
                                    start=(c == 0),
                                    stop=(c == KC - 1),
                                )
                        for m in (2 * pair, 2 * pair + 1):
                            if m % 2 == 0:
                                nc.scalar.activation(t1[:, m, :], psB[m][:], COPY)
                            else:
                                nc.vector.tensor_copy(t1[:, m, :], psB[m][:])
                        # one DMA per pair: each dma_start pays ~0.6us issue +
                        # ~2us HBM write receipt, so fewer/bigger is faster
                        nc.scalar.dma_start(
                            out=cc_in_v[1][:, 2 * pair:2 * pair + 2, :],
                            in_=t1[:, 2 * pair:2 * pair + 2, :],
                        )
                    if "AG1" not in ablate:
                        allgather(1)

                # ---- stage A (xw half, deferred): xw = fp8(x @ W_conv) ----
                # chunks 0..12 cover the AG1 flight; 13..31 are fenced into
                # the AG2 window below (more fill there, keeps PE warm for E)
                CSPLIT = 13

                def a_xw_chunks(ms, whc):
                    for m in ms:
                        psX = pspool.tile([P, D], F32, tag="ps", name=f"psX{m}_{_rep}")
                        for k in range(2):
                            nc.tensor.matmul(
                                out=psX[:],
                                lhsT=xT_sb[:, k, m * P:(m + 1) * P],
                                rhs=whc[:, k, :],
                                start=(k == 0),
                                stop=(k == 1),
                            )
                        nc.vector.tensor_copy(xw_sb[:, m, :], psX[:])

                if "A" not in ablate:
                    a_xw_chunks(range(CSPLIT), Whc_sb[:, :, D:])

                # ---- stage C (transposed): HlT = (A_T.T @ xw).T accumulation
                # psC[h] [P,512] += xw[:,c,h*128:].T @ aT[:,c,:]  (fp8, N=512)
                psC = {}
                if "C" not in ablate:
                    for h in range(2):
                        psC[h] = pspool.tile(
                            [P, RPC], F32, tag="psw", bufs=4, name=f"psC{h}_{_rep}"
                        )

                def stage_c_chunks(cs, first, last, lhs=None, off=0):
                    src = xw_sb if lhs is None else lhs
                    for c in cs:
                        for h in range(2):
                            nc.tensor.matmul(
                                out=psC[h][:],
                                lhsT=src[:, c - off, h * P:(h + 1) * P],
                                rhs=a_sb[:, c, :],
                                start=(first and c == cs[0]),
                                stop=(last and c == cs[-1]),
                            )

                if "C" not in ablate:
                    stage_c_chunks(list(range(CSPLIT)), True, False)

                # ---- stage D: P2_loc = lap[rows] @ P1, split on rb halves ----
                P1_sb = apool.tile([P, KC, D], BF16, tag="P1")
                gather_load(1, 1, P1_sb)
                gather_load(1, 2, P1_sb)
                psD = {}
                if "D" not in ablate:
                    for m in range(MT):
                        psD[m] = pspool.tile([P, D], F32, tag="ps", name=f"psD{m}_{_rep}")
                    # chunk-block-major: consume each readback piece for all m
                    # before needing the next piece; the LAST block runs
                    # m-pair-major so pair {0,1}'s staging overlaps pair
                    # {2,3}'s matmuls and the AG2 doorbell fires at D's end
                    blocks = [HALF1[:4], HALF1[4:], HALF2[:4], HALF2[4:]]
                    for bi, blk in enumerate(blocks[:3]):
                        for m in range(MT):
                            for c in blk:
                                nc.tensor.matmul(
                                    out=psD[m][:],
                                    lhsT=l_sb[:, c, m * P:(m + 1) * P],
                                    rhs=P1_sb[:, c, :],
                                    start=(bi == 0 and c == blk[0]),
                                    stop=False,
                                )
                    blk = blocks[3]
                    t2 = opool.tile([P, MT, D], BF16, tag="gst", name=f"gs2_{_rep}")
                    for pair in (0, 1):
                        for m in (2 * pair, 2 * pair + 1):
                            for c in blk:
                                nc.tensor.matmul(
                                    out=psD[m][:],
                                    lhsT=l_sb[:, c, m * P:(m + 1) * P],
                                    rhs=P1_sb[:, c, :],
                                    start=False,
                                    stop=(c == blk[-1]),
                                )
                        for m in (2 * pair, 2 * pair + 1):
                            if m % 2 == 0:
                                nc.scalar.activation(t2[:, m, :], psD[m][:], COPY)
                            else:
                                nc.vector.tensor_copy(t2[:, m, :], psD[m][:])
                        nc.scalar.dma_start(
                            out=cc_in_v[2][:, 2 * pair:2 * pair + 2, :],
                            in_=t2[:, 2 * pair:2 * pair + 2, :],
                        )
                    if "AG2" not in ablate:
                        allgather(2)

                # ---- stage C (rest): covers AG2 flight; fold bias on evacuate
                # fence: xw2 = xw[16:] + 0, where the 0 is derived from the AG2
                # staging tile -- a data dependency that stops Tile from
                # hoisting these matmuls ahead of the AG2 doorbell (v5 ran all
                # of C early, leaving the AG2 flight uncovered and E cold)
                HlT_sb = opool.tile([P, 2, RPC], BF16, tag="HlT")
                if "A" not in ablate and "D" not in ablate:
                    # fence: a zero derived from the AG2 staging tile gates a
                    # copy of W_conv; the deferred A_xw chunks read it, so
                    # Tile cannot hoist them (or the C chunks that consume
                    # their xw output) ahead of the AG2 doorbell
                    fence_t = opool.tile([P, 1], F32, tag="fence")
                    nc.vector.tensor_scalar_mul(fence_t[:], t2[:, MT - 1, :1], 0.0)
                    Whc2_sb = cpool.tile([P, 2, D], BF16, tag="Whc2")
                    nc.vector.tensor_scalar_add(
                        Whc2_sb[:], Whc_sb[:, :, D:], fence_t[:]
                    )
                    a_xw_chunks(range(CSPLIT, KC), Whc2_sb)
                elif "A" not in ablate:
                    a_xw_chunks(range(CSPLIT, KC), Whc_sb[:, :, D:])
                if "C" not in ablate:
                    stage_c_chunks(list(range(CSPLIT, KC)), False, True)
                    for h in range(2):
                        nc.scalar.activation(
                            HlT_sb[:, h, :], psC[h][:], IDENT, bias=bT_sb[:, h:h + 1]
                        )
                else:
                    for h in range(2):
                        nc.vector.memset(HlT_sb[:, h, :], 0.0)

                # ---- stage E (transposed): HhT[h] += P2[:,c,h].T @ dT[:,c,:]
                P2_sb = apool.tile([P, KC, D], BF16, tag="P2")
                gather_load(2, 1, P2_sb)
                gather_load(2, 2, P2_sb)
                if "E" not in ablate:
                    psE = {}
                    for h in range(2):
                        psE[h] = pspool.tile(
                            [P, RPC], F32, tag="psw", bufs=4, name=f"psE{h}_{_rep}"
                        )
                    # last block h-outer: h=0 finishes 12 matmuls early, so
                    # its add+store overlaps h=1's tail
                    blocks = [HALF1[:4], HALF1[4:], HALF2[:4], HALF2[4:]]
                    for bi, blk in enumerate(blocks[:3]):
                        for c in blk:
                            for h in range(2):
                                nc.tensor.matmul(
                                    out=psE[h][:],
                                    lhsT=P2_sb[:, c, h * P:(h + 1) * P],
                                    rhs=d_sb[:, c, :],
                                    start=(bi == 0 and c == blk[0]),
                                    stop=False,
                                )
                    blk = blocks[3]
                    for h in range(2):
                        for c in blk:
                            nc.tensor.matmul(
                                out=psE[h][:],
                                lhsT=P2_sb[:, c, h * P:(h + 1) * P],
                                rhs=d_sb[:, c, :],
                                start=False,
                                stop=(c == blk[-1]),
                            )
                        o_sb = opool.tile([P, RPC], BF16, tag="osb", name=f"os{h}_{_rep}")
                        nc.vector.tensor_add(o_sb[:], psE[h][:], HlT_sb[:, h, :])
                        nc.scalar.dma_start(out=out[h * P:(h + 1) * P, :], in_=o_sb[:])

    nc.finalize()
    return nc


def prep_inputs(x, edge_index, lap, d_inv, W_high, W_conv, b_conv, aL, aH):
    """Host-side sharding/layout: build per-core input maps."""
    x = np.asarray(x, dtype=np.float32)
    lap = np.asarray(lap, dtype=np.float32)
    d_inv = np.asarray(d_inv, dtype=np.float32)
    W_high = np.asarray(W_high, dtype=np.float32)
    W_conv = np.asarray(W_conv, dtype=np.float32)
    b_conv = np.asarray(b_conv, dtype=np.float32)
    aLs = float(np.asarray(aL).reshape(-1)[0])
    aHs = float(np.asarray(aH).reshape(-1)[0])
    src = np.asarray(edge_index[0], dtype=np.int64)
    dst = np.asarray(edge_index[1], dtype=np.int64)

    # symmetric GCN normalization (with self-loops) folded into a dense adjacency
    deg = np.bincount(dst, minlength=N).astype(np.float32) + 1.0
    dis = 1.0 / np.sqrt(deg)
    A_T = np.zeros((N, N), dtype=np.float32)           # A_T[src, dst]
    np.add.at(A_T, (src, dst), aLs * dis[src] * dis[dst])
    A_T[np.arange(N), np.arange(N)] += aLs * dis * dis

    def to_pkm(arrT):
        # [K, M] -> [P, kc*M]: element (p, c*M + m) = arrT[128*c + p, m]
        Kdim, Mdim = arrT.shape
        kc = Kdim // P
        a = arrT.reshape(kc, P, Mdim)
        return np.ascontiguousarray(a.transpose(1, 0, 2).reshape(P, kc * Mdim))

    xT = to_pkm(np.ascontiguousarray(x.T).astype(nbf16))
    Whc = to_pkm(np.concatenate([W_high * aHs, W_conv], axis=1).astype(nbf16))
    # bias along D (partition axis in transposed space): [P, 2] f32
    bTm = np.ascontiguousarray((aLs * b_conv).reshape(2, P).T).astype(np.float32)
    dT_full = np.ascontiguousarray(d_inv.T).astype(nbf16)
    lT_full = np.ascontiguousarray(lap.T).astype(nbf16)
    aT_full = np.clip(A_T, -240, 240).astype(nfp8)

    in_maps = []
    for i in range(NCORES):
        sl = slice(i * RPC, (i + 1) * RPC)
        in_maps.append({
            "xT": xT,
            "Whc": Whc,
            "dT": to_pkm(dT_full[:, sl]),
            "lT": to_pkm(lT_full[:, sl]),
            "aT": to_pkm(aT_full[:, sl]),
            "bT": bTm,
        })
    return in_maps


def kernel(x, edge_index, lap, d_inv, W_high, W_conv, b_conv, aL, aH):
    in_maps = prep_inputs(x, edge_index, lap, d_inv, W_high, W_conv, b_conv, aL, aH)
    nc = build_program()
    res = run_bass_kernel_spmd(nc, in_maps, list(range(NCORES)))
    # per-core output is D-major [256, 512]; transpose back to [512, 256]
    return np.concatenate(
        [res.results[i]["out"].T for i in range(NCORES)], axis=0
    ).astype(np.float32)
